# revision 36
# baseline (speedup 1.0000x reference)
"""Local-window attention encoder layer on 8 Trainium2 cores.

Problem: B=4, S=8192, D=512, window W=128, H=8 heads (HD=64), FF dim 2048.
Sharding: [B*nW]=256 independent windows split 32/core across 8 cores,
processed as 8 quads of 4 windows (512 tokens) per core.

Design notes (v2, quad-batched):
- All d-contraction GEMMs (QKV, FF1) run with free=512 over the whole quad,
  halving PE instruction count vs pair-batching (Ldweights dominates PE.SEQ).
- Scores are computed TRANSPOSED (scoresT = K^T Q per head) so the softmax
  probabilities come out already in the [kt, qt] layout attnV needs — the 8
  per-window PE transposes of probs and their PSUM evacuations are gone.
  Softmax sums (over kt = partitions) are computed with a ones[128,128]
  stationary matmul whose output replicates the per-(h,qt) sums across all
  partitions; reciprocal_approx_fast + one tensor_mul normalizes.
- Odd heads' q/k slices live at partitions 64-127 and are fed to the PE with
  tile_position=(64,0) instead of being copied down by the Act engine.
- x is cast to bf16 host-side: halves the x DMA and removes the device casts.
  Residuals add bf16 x into f32 PSUM output (verified: rel err 4.4e-3).
- Engine balance: PE does matmuls/transposes only; Act does Exp, Relu-share,
  Square(+sumsq accum), casts; DVE does residual STT(+sum accum), LN stats
  (Quake rsqrt Newton chain), LN apply, softmax recip+normalize; the
  otherwise-idle GpSimd(Pool) engine takes the big PSUM->SBUF evacuations
  (xT, lnT, qkt-share, relu-share).

Spec-driven fast path: the harness generates in_proj_b/out_b/b1/b2/ln*_b
as zeros and ln*_g as ones; nonzero/non-one values enable exact general-path
ops via build flags (rank-1 bias matmuls, gamma/beta broadcasts).
"""

import numpy as np
import ml_dtypes

import concourse.bass as bass
import concourse.tile as tile
from concourse import bacc, mybir
from concourse.bass_utils import run_bass_kernel_spmd

BF16 = ml_dtypes.bfloat16
F32 = mybir.dt.float32
I32 = mybir.dt.int32
BF = mybir.dt.bfloat16
AF = mybir.ActivationFunctionType
ALU = mybir.AluOpType
AX = mybir.AxisListType

D = 512
H = 8
W = 128
HD = 64
FF = 2048
EPS = 1e-5
N_CORES = 8
B, S = 4, 8192
NW_TOT = (B * S) // W          # 256 windows
WPC = NW_TOT // N_CORES        # 32 windows per core
QW = 4                         # windows per quad
QUADS = WPC // QW              # 8 quads per core
KC = D // 128                  # 4 contraction chunks of 128
FC = FF // 128                 # 16 ff chunks
QT = QW * W                    # 512 tokens per quad
RD = 1.0 / D
RSQRT_MAGIC = 0x5F3759DF


def _build_nc(n_quads=QUADS, flags=()):
    """flags: subset of {'qkb','vb','ob','b1','b2','gb1','gb2'} enabling
    exact handling of nonzero biases / non-unit gammas."""
    fl = set(flags)
    nc = bacc.Bacc("TRN2", target_bir_lowering=False, debug=False,
                   num_devices=N_CORES)

    x_d = nc.dram_tensor("x", [n_quads * QT, D], BF, kind="ExternalInput").ap()
    out_d = nc.dram_tensor("out", [n_quads * QT, D], F32,
                           kind="ExternalOutput").ap()
    wqk_d = nc.dram_tensor("wqk", [128, KC * 1024], BF, kind="ExternalInput").ap()
    wv_d = nc.dram_tensor("wv", [128, KC * D], BF, kind="ExternalInput").ap()
    wo_d = nc.dram_tensor("wo", [128, KC * D], BF, kind="ExternalInput").ap()
    w1_d = nc.dram_tensor("w1t", [128, KC * FF], BF, kind="ExternalInput").ap()
    w2_d = nc.dram_tensor("w2t", [128, FC * D], BF, kind="ExternalInput").ap()
    id_d = nc.dram_tensor("ident", [128, 128], BF, kind="ExternalInput").ap()
    on128_d = nc.dram_tensor("ones128", [128, 128], BF, kind="ExternalInput").ap()
    if 'b1' in fl:
        b1_d = nc.dram_tensor("b1t", [128, FC], F32, kind="ExternalInput").ap()
    if fl & {'qkb', 'vb', 'ob', 'b2'}:
        on_d = nc.dram_tensor("ones1", [1, 512], BF, kind="ExternalInput").ap()
    if 'qkb' in fl:
        qkb_d = nc.dram_tensor("qkb", [1, 1024], BF, kind="ExternalInput").ap()
    if 'vb' in fl:
        vb_d = nc.dram_tensor("vbr", [1, D], BF, kind="ExternalInput").ap()
    if 'ob' in fl:
        ob_d = nc.dram_tensor("obr", [1, D], BF, kind="ExternalInput").ap()
    if 'b2' in fl:
        b2_d = nc.dram_tensor("b2r", [1, D], BF, kind="ExternalInput").ap()
    if 'gb1' in fl:
        g1_d = nc.dram_tensor("g1b", [128, D], F32, kind="ExternalInput").ap()
        bb1_d = nc.dram_tensor("bb1", [128, D], F32, kind="ExternalInput").ap()
    if 'gb2' in fl:
        g2_d = nc.dram_tensor("g2b", [128, D], F32, kind="ExternalInput").ap()
        bb2_d = nc.dram_tensor("bb2", [128, D], F32, kind="ExternalInput").ap()

    xv = x_d.rearrange("(w p) d -> w p d", p=W)
    ov = out_d.rearrange("(w p) d -> w p d", p=W)

    with tile.TileContext(nc) as tc:
        with (
            tc.tile_pool(name="const", bufs=1) as cp,
            tc.tile_pool(name="s2", bufs=2) as sp,
            tc.tile_pool(name="s3", bufs=3) as sp3,
            tc.tile_pool(name="s4", bufs=4) as sp4,
            tc.tile_pool(name="pt", bufs=2, space="PSUM") as pt,   # 4KB tiles
            tc.tile_pool(name="p1", bufs=4, space="PSUM") as p1,   # 2KB tiles
        ):
            # ---- resident constants (DMA order interleaved with first x
            # loads below so quad 0 isn't stuck behind 6MB of weights) ----
            ident = cp.tile([128, 128], BF); nc.sync.dma_start(ident[:], id_d[:])
            on128 = cp.tile([128, 128], BF); nc.sync.dma_start(on128[:], on128_d[:])
            wqk = cp.tile([128, KC, 1024], BF)
            wv = cp.tile([128, KC, D], BF)
            wo = cp.tile([128, KC, D], BF)
            w1t = cp.tile([128, KC, FF], BF)
            w2t = cp.tile([128, FC, D], BF)
            magic = cp.tile([128, 8], I32)
            nc.vector.memset(magic[:, 0:4], RSQRT_MAGIC)
            nc.vector.memset(magic[:, 4:8], 1)
            if 'b1' in fl:
                b1t = cp.tile([128, FC], F32); nc.sync.dma_start(b1t[:], b1_d[:])
            if fl & {'qkb', 'vb', 'ob', 'b2'}:
                ones1 = cp.tile([1, 512], BF); nc.sync.dma_start(ones1[:], on_d[:])
            if 'qkb' in fl:
                qkb = cp.tile([1, 1024], BF); nc.sync.dma_start(qkb[:], qkb_d[:])
            if 'vb' in fl:
                vbr = cp.tile([1, D], BF); nc.sync.dma_start(vbr[:], vb_d[:])
            if 'ob' in fl:
                obr = cp.tile([1, D], BF); nc.sync.dma_start(obr[:], ob_d[:])
            if 'b2' in fl:
                b2r = cp.tile([1, D], BF); nc.sync.dma_start(b2r[:], b2_d[:])
            if 'gb1' in fl:
                g1b = cp.tile([128, D], F32); nc.sync.dma_start(g1b[:], g1_d[:])
                bb1 = cp.tile([128, D], F32); nc.sync.dma_start(bb1[:], bb1_d[:])
            if 'gb2' in fl:
                g2b = cp.tile([128, D], F32); nc.sync.dma_start(g2b[:], g2_d[:])
                bb2 = cp.tile([128, D], F32); nc.sync.dma_start(bb2[:], bb2_d[:])

            def ln_stats(st, off=0, width=QW):
                """LN stats for `width` windows starting at column `off`:
                cols 0-3 sum, 4-7 sumsq -> rstd cols 32-35, -mu*rstd 36-39."""
                c = lambda a: st[:, a + off:a + off + width]
                ve = nc.vector
                ve.tensor_scalar_mul(c(8), c(0), -RD)       # -mu
                ve.tensor_mul(c(12), c(8), c(8))            # mu^2
                ve.tensor_scalar_mul(c(16), c(4), RD)       # E[y^2]
                ve.tensor_sub(c(20), c(16), c(12))          # var
                ve.tensor_scalar_add(c(24), c(20), EPS)     # q
                q = c(24)
                mg = lambda a: magic[:, a + off:a + off + width]
                ve.tensor_tensor(c(28).bitcast(I32), q.bitcast(I32),
                                 mg(4), ALU.logical_shift_right)
                ve.tensor_tensor(c(40).bitcast(I32), mg(0),
                                 c(28).bitcast(I32), ALU.subtract)
                r0 = c(40)
                ve.tensor_mul(c(44), r0, r0)
                ve.tensor_mul(c(48), c(44), q)
                ve.tensor_scalar(c(52), c(48), -0.5, 1.5,
                                 ALU.mult, ALU.add)
                ve.tensor_mul(c(56), r0, c(52))
                r1 = c(56)
                ve.tensor_mul(c(44), r1, r1)
                ve.tensor_mul(c(48), c(44), q)
                ve.tensor_scalar(c(52), c(48), -0.5, 1.5,
                                 ALU.mult, ALU.add)
                ve.tensor_mul(c(32), r1, c(52))             # rstd
                ve.tensor_mul(c(36), c(8), c(32))           # -mu*rstd

            def ln_apply(o, y, st, w, gb, eng):
                """o = y*rstd + (-mu*rstd) [+ gamma/beta], engine-selectable."""
                if eng == 'act':
                    nc.scalar.activation(o, y, AF.Identity,
                                         bias=st[:, 36 + w:37 + w],
                                         scale=st[:, 32 + w:33 + w])
                elif eng == 'pool':
                    nc.gpsimd.tensor_scalar(o, y, st[:, 32 + w:33 + w],
                                            st[:, 36 + w:37 + w],
                                            ALU.mult, ALU.add)
                else:
                    nc.vector.tensor_scalar(o, y, st[:, 32 + w:33 + w],
                                            st[:, 36 + w:37 + w],
                                            ALU.mult, ALU.add)
                if gb is not None:
                    g, b = gb
                    nc.vector.tensor_mul(o, o, g[:])
                    nc.vector.tensor_add(o, o, b[:])

            def load(q):
                """emit x DMAs for quad q -> one [128, QW, D] bf16 tile."""
                x = sp3.tile([128, QW, D], BF, tag="x")
                for w in range(QW):
                    nc.sync.dma_start(x[:, w, :], xv[QW * q + w])
                return x

            def xT(x):
                """transpose a quad's x into [d, token] layout (16 PE
                transposes, DVE-evacuated per k-chunk)."""
                xts = pt.tile([128, KC, QW, 128], BF, tag="t")
                xtp = sp.tile([128, KC, QT], BF, tag="xtp")
                for k in range(KC):
                    for w in range(QW):
                        nc.tensor.transpose(xts[:, k, w, :],
                                            x[:, w, k * 128:(k + 1) * 128],
                                            ident[:])
                    nc.vector.tensor_copy(xtp[:, k, :], xts[:, k, :, :])
                return xtp

            def front_a(q, x, xtp, x_next):
                """QKV for quad q + transpose of quad q+1's x. The FF1 of
                quad q-1 is emitted right after this stage, giving the PE
                ~14us of independent work while DVE/Act evacuate qkt/v."""
                # ---- QK: 8 out-blocks, free=512 ----
                qkt = sp.tile([128, 8, QT], BF, tag="qkt")
                for e in range(8):
                    pq = p1.tile([128, QT], F32, tag="m")
                    for k in range(KC):
                        nc.tensor.matmul(
                            pq[:], wqk[:, k, e * 128:(e + 1) * 128],
                            xtp[:, k, :], start=(k == 0),
                            stop=(k == KC - 1 and 'qkb' not in fl))
                    if 'qkb' in fl:
                        nc.tensor.matmul(
                            pq[:], qkb[:, e * 128:(e + 1) * 128],
                            ones1[:, 0:QT], start=False, stop=True)
                    if e % 2 == 0:
                        nc.vector.tensor_copy(qkt[:, e, :], pq[:])
                    else:
                        nc.scalar.copy(qkt[:, e, :], pq[:])

                # ---- V: per window, free=512 ----
                vt = sp.tile([128, QW, D], BF, tag="vt")
                for w in range(QW):
                    pv = p1.tile([128, D], F32, tag="m")
                    for k in range(KC):
                        nc.tensor.matmul(
                            pv[:], xtp[:, k, w * W:(w + 1) * W], wv[:, k, :],
                            start=(k == 0),
                            stop=(k == KC - 1 and 'vb' not in fl))
                    if 'vb' in fl:
                        nc.tensor.matmul(pv[:], ones1[:, 0:128], vbr[:],
                                         start=False, stop=True)
                    nc.scalar.copy(vt[:, w, :], pv[:])

                xtp_next = xT(x_next) if x_next is not None else None
                return {"q": q, "x": x, "qkt": qkt, "vt": vt,
                        "xtp_next": xtp_next}

            def front_b(fa, prev):
                """scoresT, softmax, attnV, out-proj, LN1 for quad q, with
                quad q-1's FF1 interleaved at window granularity."""
                q, x, qkt, vt = fa["q"], fa["x"], fa["qkt"], fa["vt"]
                prw, smw = [], []

                # Head order is parity-major (0,2,4,6,1,3,5,7) so the even
                # heads' row-tile T0 writes only PSUM bank 0 and the odd
                # heads' T8 only bank 1 — row tiles must not touch the same
                # PSUM bank concurrently.
                def sT_block(w):
                    psc = pt.tile([128, H, 128], F32, tag="t")
                    for i in range(H):
                        h = 2 * (i % 4) + i // 4    # slot i holds head h
                        pb = (i // 4) * 64
                        lq = qkt[pb:pb + 64, h // 2, w * W:(w + 1) * W]
                        lk = qkt[pb:pb + 64, 4 + h // 2, w * W:(w + 1) * W]
                        nc.tensor.matmul(psc[:, i, :], lk, lq, start=True,
                                         stop=True, tile_position=(pb, 0))
                    pr = sp4.tile([128, H, 128], BF, tag="pr")
                    nc.scalar.activation(pr[:], psc[:], AF.Exp)
                    prw.append(pr)

                def sums_block(w):
                    # Column-tiled sums: partitions 0-63 get the even heads'
                    # (slots 0-3) sums, 64-127 the odd heads' — matching the
                    # packed attnT layout, so one [128,512] reciprocal (half
                    # the elements) scales the evacuation directly.
                    pr = prw[w]
                    sm = p1.tile([128, QW, 128], F32, tag="m")
                    nc.tensor.matmul(sm[0:64, :, :], on128[:, 0:64],
                                     pr[:, 0:4, :], start=True, stop=True,
                                     tile_position=(0, 0))
                    nc.tensor.matmul(sm[64:128, :, :], on128[:, 0:64],
                                     pr[:, 4:8, :], start=True, stop=True,
                                     tile_position=(0, 64))
                    smw.append(sm)

                def attn_pv(w):
                    # attnV on the UNNORMALIZED exp scores; the softmax
                    # reciprocal is folded into the PSUM evacuation (attn_out)
                    # so the PE never waits on recip.
                    pr, sm = prw[w], smw[w]
                    rcp = sp.tile([128, QW, 128], F32, tag="rcp")
                    nc.vector.reciprocal_approx_fast(out=rcp[:], in_=sm[:])
                    pat = p1.tile([128, D], F32, tag="m")
                    for h in range(H):
                        pb = (h % 2) * 64
                        slot = (h % 2) * 4 + h // 2   # pr slot of head h
                        nc.tensor.matmul(
                            pat[pb:pb + 64, (h // 2) * 128:(h // 2 + 1) * 128],
                            vt[:, w, h * HD:(h + 1) * HD], pr[:, slot, :],
                            start=True, stop=True, tile_position=(0, pb))
                    return rcp, pat

                def attn_out(w, rcp, pat):
                    ats = sp4.tile([128, D], BF, tag="ats")
                    nc.vector.tensor_mul(ats[:], pat[:], rcp[:])
                    pao = p1.tile([128, D], F32, tag="m")
                    for k in range(KC):
                        nc.tensor.matmul(pao[:], ats[:, k * 128:(k + 1) * 128],
                                         wo[:, k, :], start=(k == 0),
                                         stop=(k == KC - 1 and 'ob' not in fl))
                    if 'ob' in fl:
                        nc.tensor.matmul(pao[:], ones1[:, 0:128], obr[:],
                                         start=False, stop=True)
                    return pao

                # Per-window software pipeline: scoresT runs 2 windows ahead
                # of consumption, sums 1 ahead, and quad q-1's FF1 blocks are
                # interleaved as PE filler (cursor-scheduled) so the array
                # never waits on the exp/recip/evacuation/residual chains.
                ffc = [0]

                def ff1_take(n, eng='mix'):
                    if prev is not None and ffc[0] < FC:
                        m1 = min(ffc[0] + n, FC)
                        ff1_blocks(prev, ffc[0], m1, eng)
                        ffc[0] = m1

                sT_block(0); sT_block(1)
                ff1_take(4)            # cover exp(w0)/qkt evacs before sums
                sums_block(0)

                st1 = sp.tile([128, 64], F32, tag="st1")
                y1w = []
                sq = sp.tile([128, D], BF, tag="sq")
                ln1 = sp3.tile([128, QW, D], BF, tag="ln1")
                for w in range(QW):
                    rcp, pat = attn_pv(w)
                    if w + 2 < QW:
                        sT_block(w + 2)
                    if w + 1 < QW:
                        sums_block(w + 1)
                    ff1_take(1)
                    pao = attn_out(w, rcp, pat)
                    y1 = sp4.tile([128, D], F32, tag="y1")
                    nc.vector.scalar_tensor_tensor(
                        y1[:], pao[:], 0.0, x[:, w, :], ALU.add, ALU.add,
                        accum_out=st1[:, w:w + 1])
                    nc.scalar.activation(sq[:], y1[:], AF.Square,
                                         accum_out=st1[:, QW + w:QW + w + 1])
                    y1w.append(y1)
                    ff1_take(2)
                ln_stats(st1)
                gb = (g1b, bb1) if 'gb1' in fl else None
                for w in range(QW):
                    ln_apply(ln1[:, w, :], y1w[w][:], st1, w, gb, 'pool')
                return {"q": q, "ln1": ln1, "xtp_next": fa["xtp_next"]}

            def lnT(state):
                """transpose LN1 output for the FF stage (emitted a full
                stage after LN1-apply so the PE never waits on it)."""
                ln1 = state["ln1"]
                lts = pt.tile([128, KC, QW, 128], BF, tag="t")
                lnt = sp.tile([128, KC, QT], BF, tag="lnt")
                for k in range(KC):
                    for w in range(QW):
                        nc.tensor.transpose(lts[:, k, w, :],
                                            ln1[:, w, k * 128:(k + 1) * 128],
                                            ident[:])
                    nc.vector.tensor_copy(lnt[:, k, :], lts[:, k, :, :])
                state["lnt"] = lnt

            def ff1_blocks(state, m0, m1, eng='mix'):
                """FF1 f-blocks [m0, m1) + relu for quad `state` (uses lnt
                from the lnT stage). `eng` picks the relu engine: the
                mid-window pair goes to Act (its queue is short there, and
                the next window's attnV PSUM allocation WAR-waits on that
                relu via the ring), the end pair to DVE."""
                lnt = state["lnt"]
                if "h1r" not in state:
                    h1r = sp.tile([128, FC, QT], BF, tag="h1r")
                    state["h1r"] = h1r
                h1r = state["h1r"]
                for m in range(m0, m1):
                    ph = p1.tile([128, QT], F32, tag="m")
                    for k in range(KC):
                        nc.tensor.matmul(
                            ph[:], w1t[:, k, m * 128:(m + 1) * 128],
                            lnt[:, k, :], start=(k == 0), stop=(k == KC - 1))
                    use_act = (m % 2 == 1) if eng == 'mix' else (eng == 'act')
                    bm = b1t[:, m:m + 1] if 'b1' in fl else None
                    if use_act:
                        if bm is not None:
                            nc.scalar.activation(h1r[:, m, :], ph[:], AF.Relu,
                                                 bias=bm)
                        else:
                            nc.scalar.activation(h1r[:, m, :], ph[:], AF.Relu)
                    else:
                        if bm is not None:
                            nc.vector.tensor_scalar(h1r[:, m, :], ph[:], bm,
                                                    0.0, ALU.add, ALU.max)
                        else:
                            nc.vector.tensor_scalar(h1r[:, m, :], ph[:], 0.0,
                                                    None, ALU.max)

            def back_b(state, last=False):
                """FF2, LN2, store for quad q. On the final quad the LN2
                applies run on DVE/Act (faster + parallel) instead of Pool:
                there is no PE work left to hide the tail behind."""
                q, ln1, h1r = state["q"], state["ln1"], state["h1r"]
                st2 = sp.tile([128, 64], F32, tag="st2")
                y2w = []
                sq2 = sp.tile([128, D], BF, tag="sq2")
                for w in range(QW):
                    pf = p1.tile([128, D], F32, tag="m")
                    for m in range(FC):
                        nc.tensor.matmul(
                            pf[:], h1r[:, m, w * W:(w + 1) * W],
                            w2t[:, m, :], start=(m == 0),
                            stop=(m == FC - 1 and 'b2' not in fl))
                    if 'b2' in fl:
                        nc.tensor.matmul(pf[:], ones1[:, 0:128], b2r[:],
                                         start=False, stop=True)
                    y2 = sp4.tile([128, D], F32, tag="y2")
                    nc.vector.scalar_tensor_tensor(
                        y2[:], pf[:], 0.0, ln1[:, w, :], ALU.add, ALU.add,
                        accum_out=st2[:, w:w + 1])
                    nc.scalar.activation(sq2[:], y2[:], AF.Square,
                                         accum_out=st2[:, QW + w:QW + w + 1])
                    y2w.append(y2)
                    if last:
                        # per-window stats+apply+store: each window's LN2
                        # chain hides under the next window's FF2 matmuls
                        # instead of piling up after the last one.
                        ln_stats(st2, off=w, width=1)
                        yo = sp.tile([128, D], F32, tag="yo")
                        ln_apply(yo[:], y2[:], st2, w,
                                 (g2b, bb2) if 'gb2' in fl else None,
                                 'act' if w % 2 else 'dve')
                        if w % 2:
                            nc.gpsimd.dma_start(ov[QW * q + w], yo[:])
                        else:
                            nc.sync.dma_start(ov[QW * q + w], yo[:])
                if last:
                    return
                ln_stats(st2)
                gb = (g2b, bb2) if 'gb2' in fl else None
                for w in range(QW):
                    yo = sp.tile([128, D], F32, tag="yo")
                    ln_apply(yo[:], y2w[w][:], st2, w, gb, 'pool')
                    nc.sync.dma_start(ov[QW * q + w], yo[:])

            xq = [load(0)]
            for k in range(KC):
                nc.sync.dma_start(wqk[:, k, :], wqk_d[:, k * 1024:(k + 1) * 1024])
            nc.sync.dma_start(wv[:], wv_d[:])
            nc.sync.dma_start(wo[:], wo_d[:])
            if n_quads > 1:
                xq.append(load(1))
            nc.sync.dma_start(w1t[:], w1_d[:])
            nc.sync.dma_start(w2t[:], w2_d[:])
            # Pipeline rotation: front_a(q+1) is emitted between back_b(q-1)
            # and lnT(q), so the LN1-apply chain of quad q has back_b + a full
            # QKV stage (~25us of PE work) of cover before the lnT transposes
            # need it.
            xtp_cur = xT(xq[0])
            fa = front_a(0, xq[0], xtp_cur,
                         xq[1] if n_quads > 1 else None)
            pending = None
            for q in range(n_quads):
                nxt = front_b(fa, pending)
                if q + 2 < n_quads:
                    xq.append(load(q + 2))
                if pending is not None:
                    back_b(pending)
                if q + 1 < n_quads:
                    x_next = xq[q + 2] if q + 2 < n_quads else None
                    fa = front_a(q + 1, xq[q + 1], fa["xtp_next"], x_next)
                lnT(nxt)
                pending = nxt
            ff1_blocks(pending, 0, FC)
            back_b(pending, last=True)

    nc.compile()
    return nc


def _pack(wT, kc):
    """[kc*128, N] -> [128, kc*N] with partition p, block k = wT[k*128+p]."""
    n = wT.shape[1]
    return np.ascontiguousarray(
        wT.reshape(kc, 128, n).transpose(1, 0, 2).reshape(128, kc * n))


_CACHE = {}


def _get_nc(n_quads=QUADS, flags=()):
    key = (n_quads, tuple(sorted(flags)))
    if key not in _CACHE:
        _CACHE[key] = _build_nc(n_quads, flags)
    return _CACHE[key]


def _flags(in_proj_b, out_b, ln1_g, ln1_b, b1, b2, ln2_g, ln2_b):
    fl = []
    in_proj_b = np.asarray(in_proj_b)
    if np.any(in_proj_b[:2 * D]):
        fl.append('qkb')
    if np.any(in_proj_b[2 * D:]):
        fl.append('vb')
    if np.any(np.asarray(out_b)):
        fl.append('ob')
    if np.any(np.asarray(b1)):
        fl.append('b1')
    if np.any(np.asarray(b2)):
        fl.append('b2')
    if np.any(np.asarray(ln1_b)) or not np.all(np.asarray(ln1_g) == 1.0):
        fl.append('gb1')
    if np.any(np.asarray(ln2_b)) or not np.all(np.asarray(ln2_g) == 1.0):
        fl.append('gb2')
    return tuple(fl)


def _prep_inputs(src, in_proj_w, in_proj_b, out_w, out_b, ln1_g, ln1_b,
                 w1, b1, w2, b2, ln2_g, ln2_b, n_quads=QUADS, flags=()):
    fl = set(flags)
    src = np.asarray(src, np.float32)
    scale = 1.0 / np.sqrt(HD)

    wqkT = np.asarray(in_proj_w[:2 * D], np.float32).T.copy()   # [512, 1024]
    wqkT[:, :D] *= scale

    common = {
        "wqk": _pack(wqkT.astype(BF16), KC),
        "wv": _pack(np.asarray(in_proj_w[2 * D:], np.float32).T.astype(BF16), KC),
        "wo": _pack(np.asarray(out_w, np.float32).T.astype(BF16), KC),
        "w1t": _pack(np.asarray(w1, np.float32).T.astype(BF16), KC),
        "w2t": _pack(np.asarray(w2, np.float32).T.astype(BF16), FC),
        "ident": np.eye(128, dtype=BF16),
        "ones128": np.ones((128, 128), BF16),
    }
    if 'b1' in fl:
        common["b1t"] = np.ascontiguousarray(
            np.asarray(b1, np.float32).reshape(FC, 128).T)
    if fl & {'qkb', 'vb', 'ob', 'b2'}:
        common["ones1"] = np.ones((1, 512), BF16)
    if 'qkb' in fl:
        bqk = np.asarray(in_proj_b[:2 * D], np.float32).copy()
        bqk[:D] *= scale
        common["qkb"] = bqk.astype(BF16)[None, :]
    if 'vb' in fl:
        common["vbr"] = np.asarray(in_proj_b[2 * D:], np.float32).astype(BF16)[None, :]
    if 'ob' in fl:
        common["obr"] = np.asarray(out_b, np.float32).astype(BF16)[None, :]
    if 'b2' in fl:
        common["b2r"] = np.asarray(b2, np.float32).astype(BF16)[None, :]
    if 'gb1' in fl:
        common["g1b"] = np.ascontiguousarray(np.broadcast_to(
            np.asarray(ln1_g, np.float32), (128, D)))
        common["bb1"] = np.ascontiguousarray(np.broadcast_to(
            np.asarray(ln1_b, np.float32), (128, D)))
    if 'gb2' in fl:
        common["g2b"] = np.ascontiguousarray(np.broadcast_to(
            np.asarray(ln2_g, np.float32), (128, D)))
        common["bb2"] = np.ascontiguousarray(np.broadcast_to(
            np.asarray(ln2_b, np.float32), (128, D)))

    wins = src.reshape(NW_TOT, W, D)
    wpc = n_quads * QW
    in_maps = []
    for c in range(N_CORES):
        m = dict(common)
        m["x"] = np.ascontiguousarray(
            wins[c * wpc:(c + 1) * wpc].reshape(wpc * W, D)).astype(BF16)
        in_maps.append(m)
    return in_maps


def kernel(src, in_proj_w, in_proj_b, out_w, out_b, ln1_g, ln1_b,
           w1, b1, w2, b2, ln2_g, ln2_b):
    flags = _flags(in_proj_b, out_b, ln1_g, ln1_b, b1, b2, ln2_g, ln2_b)
    nc = _get_nc(QUADS, flags)
    in_maps = _prep_inputs(src, in_proj_w, in_proj_b, out_w, out_b, ln1_g,
                           ln1_b, w1, b1, w2, b2, ln2_g, ln2_b, QUADS, flags)
    res = run_bass_kernel_spmd(nc, in_maps, list(range(N_CORES)))
    out = np.concatenate([res.results[c]["out"] for c in range(N_CORES)], axis=0)
    return np.ascontiguousarray(out.reshape(B, S, D)).astype(np.float32)


# revision 37
# speedup vs baseline: 1.0019x; 1.0019x over previous
"""Local-window attention encoder layer on 8 Trainium2 cores.

Problem: B=4, S=8192, D=512, window W=128, H=8 heads (HD=64), FF dim 2048.
Sharding: [B*nW]=256 independent windows split 32/core across 8 cores,
processed as 8 quads of 4 windows (512 tokens) per core.

Design notes (v2, quad-batched):
- All d-contraction GEMMs (QKV, FF1) run with free=512 over the whole quad,
  halving PE instruction count vs pair-batching (Ldweights dominates PE.SEQ).
- Scores are computed TRANSPOSED (scoresT = K^T Q per head) so the softmax
  probabilities come out already in the [kt, qt] layout attnV needs — the 8
  per-window PE transposes of probs and their PSUM evacuations are gone.
  Softmax sums (over kt = partitions) are computed with a ones[128,128]
  stationary matmul whose output replicates the per-(h,qt) sums across all
  partitions; reciprocal_approx_fast + one tensor_mul normalizes.
- Odd heads' q/k slices live at partitions 64-127 and are fed to the PE with
  tile_position=(64,0) instead of being copied down by the Act engine.
- x is cast to bf16 host-side: halves the x DMA and removes the device casts.
  Residuals add bf16 x into f32 PSUM output (verified: rel err 4.4e-3).
- Engine balance: PE does matmuls/transposes only; Act does Exp, Relu-share,
  Square(+sumsq accum), casts; DVE does residual STT(+sum accum), LN stats
  (Quake rsqrt Newton chain), LN apply, softmax recip+normalize; the
  otherwise-idle GpSimd(Pool) engine takes the big PSUM->SBUF evacuations
  (xT, lnT, qkt-share, relu-share).

Spec-driven fast path: the harness generates in_proj_b/out_b/b1/b2/ln*_b
as zeros and ln*_g as ones; nonzero/non-one values enable exact general-path
ops via build flags (rank-1 bias matmuls, gamma/beta broadcasts).
"""

import numpy as np
import ml_dtypes

import concourse.bass as bass
import concourse.tile as tile
from concourse import bacc, mybir
from concourse.bass_utils import run_bass_kernel_spmd

BF16 = ml_dtypes.bfloat16
F32 = mybir.dt.float32
I32 = mybir.dt.int32
BF = mybir.dt.bfloat16
AF = mybir.ActivationFunctionType
ALU = mybir.AluOpType
AX = mybir.AxisListType

D = 512
H = 8
W = 128
HD = 64
FF = 2048
EPS = 1e-5
N_CORES = 8
B, S = 4, 8192
NW_TOT = (B * S) // W          # 256 windows
WPC = NW_TOT // N_CORES        # 32 windows per core
QW = 4                         # windows per quad
QUADS = WPC // QW              # 8 quads per core
KC = D // 128                  # 4 contraction chunks of 128
FC = FF // 128                 # 16 ff chunks
QT = QW * W                    # 512 tokens per quad
RD = 1.0 / D
RSQRT_MAGIC = 0x5F3759DF


def _build_nc(n_quads=QUADS, flags=()):
    """flags: subset of {'qkb','vb','ob','b1','b2','gb1','gb2'} enabling
    exact handling of nonzero biases / non-unit gammas."""
    fl = set(flags)
    nc = bacc.Bacc("TRN2", target_bir_lowering=False, debug=False,
                   num_devices=N_CORES)

    x_d = nc.dram_tensor("x", [n_quads * QT, D], BF, kind="ExternalInput").ap()
    out_d = nc.dram_tensor("out", [n_quads * QT, D], F32,
                           kind="ExternalOutput").ap()
    wqk_d = nc.dram_tensor("wqk", [128, KC * 1024], BF, kind="ExternalInput").ap()
    wv_d = nc.dram_tensor("wv", [128, KC * D], BF, kind="ExternalInput").ap()
    wo_d = nc.dram_tensor("wo", [128, KC * D], BF, kind="ExternalInput").ap()
    w1_d = nc.dram_tensor("w1t", [128, KC * FF], BF, kind="ExternalInput").ap()
    w2_d = nc.dram_tensor("w2t", [128, FC * D], BF, kind="ExternalInput").ap()
    id_d = nc.dram_tensor("ident", [128, 128], BF, kind="ExternalInput").ap()
    on128_d = nc.dram_tensor("ones128", [128, 128], BF, kind="ExternalInput").ap()
    if 'b1' in fl:
        b1_d = nc.dram_tensor("b1t", [128, FC], F32, kind="ExternalInput").ap()
    if fl & {'qkb', 'vb', 'ob', 'b2'}:
        on_d = nc.dram_tensor("ones1", [1, 512], BF, kind="ExternalInput").ap()
    if 'qkb' in fl:
        qkb_d = nc.dram_tensor("qkb", [1, 1024], BF, kind="ExternalInput").ap()
    if 'vb' in fl:
        vb_d = nc.dram_tensor("vbr", [1, D], BF, kind="ExternalInput").ap()
    if 'ob' in fl:
        ob_d = nc.dram_tensor("obr", [1, D], BF, kind="ExternalInput").ap()
    if 'b2' in fl:
        b2_d = nc.dram_tensor("b2r", [1, D], BF, kind="ExternalInput").ap()
    if 'gb1' in fl:
        g1_d = nc.dram_tensor("g1b", [128, D], F32, kind="ExternalInput").ap()
        bb1_d = nc.dram_tensor("bb1", [128, D], F32, kind="ExternalInput").ap()
    if 'gb2' in fl:
        g2_d = nc.dram_tensor("g2b", [128, D], F32, kind="ExternalInput").ap()
        bb2_d = nc.dram_tensor("bb2", [128, D], F32, kind="ExternalInput").ap()

    xv = x_d.rearrange("(w p) d -> w p d", p=W)
    ov = out_d.rearrange("(w p) d -> w p d", p=W)

    with tile.TileContext(nc) as tc:
        with (
            tc.tile_pool(name="const", bufs=1) as cp,
            tc.tile_pool(name="s2", bufs=2) as sp,
            tc.tile_pool(name="s3", bufs=3) as sp3,
            tc.tile_pool(name="s4", bufs=4) as sp4,
            tc.tile_pool(name="pt", bufs=2, space="PSUM") as pt,   # 4KB tiles
            tc.tile_pool(name="p1", bufs=4, space="PSUM") as p1,   # 2KB tiles
        ):
            # ---- resident constants (DMA order interleaved with first x
            # loads below so quad 0 isn't stuck behind 6MB of weights) ----
            ident = cp.tile([128, 128], BF); nc.sync.dma_start(ident[:], id_d[:])
            on128 = cp.tile([128, 128], BF)
            wqk = cp.tile([128, KC, 1024], BF)
            wv = cp.tile([128, KC, D], BF)
            wo = cp.tile([128, KC, D], BF)
            w1t = cp.tile([128, KC, FF], BF)
            w2t = cp.tile([128, FC, D], BF)
            magic = cp.tile([128, 8], I32)
            nc.vector.memset(magic[:, 0:4], RSQRT_MAGIC)
            nc.vector.memset(magic[:, 4:8], 1)
            if 'b1' in fl:
                b1t = cp.tile([128, FC], F32); nc.sync.dma_start(b1t[:], b1_d[:])
            if fl & {'qkb', 'vb', 'ob', 'b2'}:
                ones1 = cp.tile([1, 512], BF); nc.sync.dma_start(ones1[:], on_d[:])
            if 'qkb' in fl:
                qkb = cp.tile([1, 1024], BF); nc.sync.dma_start(qkb[:], qkb_d[:])
            if 'vb' in fl:
                vbr = cp.tile([1, D], BF); nc.sync.dma_start(vbr[:], vb_d[:])
            if 'ob' in fl:
                obr = cp.tile([1, D], BF); nc.sync.dma_start(obr[:], ob_d[:])
            if 'b2' in fl:
                b2r = cp.tile([1, D], BF); nc.sync.dma_start(b2r[:], b2_d[:])
            if 'gb1' in fl:
                g1b = cp.tile([128, D], F32); nc.sync.dma_start(g1b[:], g1_d[:])
                bb1 = cp.tile([128, D], F32); nc.sync.dma_start(bb1[:], bb1_d[:])
            if 'gb2' in fl:
                g2b = cp.tile([128, D], F32); nc.sync.dma_start(g2b[:], g2_d[:])
                bb2 = cp.tile([128, D], F32); nc.sync.dma_start(bb2[:], bb2_d[:])

            def ln_stats(st, off=0, width=QW):
                """LN stats for `width` windows starting at column `off`:
                cols 0-3 sum, 4-7 sumsq -> rstd cols 32-35, -mu*rstd 36-39."""
                c = lambda a: st[:, a + off:a + off + width]
                ve = nc.vector
                ve.tensor_scalar_mul(c(8), c(0), -RD)       # -mu
                ve.tensor_mul(c(12), c(8), c(8))            # mu^2
                ve.tensor_scalar_mul(c(16), c(4), RD)       # E[y^2]
                ve.tensor_sub(c(20), c(16), c(12))          # var
                ve.tensor_scalar_add(c(24), c(20), EPS)     # q
                q = c(24)
                mg = lambda a: magic[:, a + off:a + off + width]
                ve.tensor_tensor(c(28).bitcast(I32), q.bitcast(I32),
                                 mg(4), ALU.logical_shift_right)
                ve.tensor_tensor(c(40).bitcast(I32), mg(0),
                                 c(28).bitcast(I32), ALU.subtract)
                r0 = c(40)
                ve.tensor_mul(c(44), r0, r0)
                ve.tensor_mul(c(48), c(44), q)
                ve.tensor_scalar(c(52), c(48), -0.5, 1.5,
                                 ALU.mult, ALU.add)
                ve.tensor_mul(c(56), r0, c(52))
                r1 = c(56)
                ve.tensor_mul(c(44), r1, r1)
                ve.tensor_mul(c(48), c(44), q)
                ve.tensor_scalar(c(52), c(48), -0.5, 1.5,
                                 ALU.mult, ALU.add)
                ve.tensor_mul(c(32), r1, c(52))             # rstd
                ve.tensor_mul(c(36), c(8), c(32))           # -mu*rstd

            def ln_apply(o, y, st, w, gb, eng):
                """o = y*rstd + (-mu*rstd) [+ gamma/beta], engine-selectable."""
                if eng == 'act':
                    nc.scalar.activation(o, y, AF.Identity,
                                         bias=st[:, 36 + w:37 + w],
                                         scale=st[:, 32 + w:33 + w])
                elif eng == 'pool':
                    nc.gpsimd.tensor_scalar(o, y, st[:, 32 + w:33 + w],
                                            st[:, 36 + w:37 + w],
                                            ALU.mult, ALU.add)
                else:
                    nc.vector.tensor_scalar(o, y, st[:, 32 + w:33 + w],
                                            st[:, 36 + w:37 + w],
                                            ALU.mult, ALU.add)
                if gb is not None:
                    g, b = gb
                    nc.vector.tensor_mul(o, o, g[:])
                    nc.vector.tensor_add(o, o, b[:])

            def load(q):
                """emit x DMAs for quad q -> one [128, QW, D] bf16 tile."""
                x = sp3.tile([128, QW, D], BF, tag="x")
                for w in range(QW):
                    nc.sync.dma_start(x[:, w, :], xv[QW * q + w])
                return x

            def xT(x):
                """transpose a quad's x into [d, token] layout (16 PE
                transposes, DVE-evacuated per k-chunk)."""
                xts = pt.tile([128, KC, QW, 128], BF, tag="t")
                xtp = sp.tile([128, KC, QT], BF, tag="xtp")
                for k in range(KC):
                    for w in range(QW):
                        nc.tensor.transpose(xts[:, k, w, :],
                                            x[:, w, k * 128:(k + 1) * 128],
                                            ident[:])
                    nc.vector.tensor_copy(xtp[:, k, :], xts[:, k, :, :])
                return xtp

            def front_a(q, x, xtp, x_next):
                """QKV for quad q + transpose of quad q+1's x. The FF1 of
                quad q-1 is emitted right after this stage, giving the PE
                ~14us of independent work while DVE/Act evacuate qkt/v."""
                # ---- QK: 8 out-blocks, free=512 ----
                qkt = sp.tile([128, 8, QT], BF, tag="qkt")
                for e in range(8):
                    pq = p1.tile([128, QT], F32, tag="m")
                    for k in range(KC):
                        nc.tensor.matmul(
                            pq[:], wqk[:, k, e * 128:(e + 1) * 128],
                            xtp[:, k, :], start=(k == 0),
                            stop=(k == KC - 1 and 'qkb' not in fl))
                    if 'qkb' in fl:
                        nc.tensor.matmul(
                            pq[:], qkb[:, e * 128:(e + 1) * 128],
                            ones1[:, 0:QT], start=False, stop=True)
                    if e % 2 == 0:
                        nc.vector.tensor_copy(qkt[:, e, :], pq[:])
                    else:
                        nc.scalar.copy(qkt[:, e, :], pq[:])

                # ---- V: per window, free=512 ----
                vt = sp.tile([128, QW, D], BF, tag="vt")
                for w in range(QW):
                    pv = p1.tile([128, D], F32, tag="m")
                    for k in range(KC):
                        nc.tensor.matmul(
                            pv[:], xtp[:, k, w * W:(w + 1) * W], wv[:, k, :],
                            start=(k == 0),
                            stop=(k == KC - 1 and 'vb' not in fl))
                    if 'vb' in fl:
                        nc.tensor.matmul(pv[:], ones1[:, 0:128], vbr[:],
                                         start=False, stop=True)
                    nc.scalar.copy(vt[:, w, :], pv[:])

                xtp_next = xT(x_next) if x_next is not None else None
                return {"q": q, "x": x, "qkt": qkt, "vt": vt,
                        "xtp_next": xtp_next}

            def front_b(fa, prev):
                """scoresT, softmax, attnV, out-proj, LN1 for quad q, with
                quad q-1's FF1 interleaved at window granularity."""
                q, x, qkt, vt = fa["q"], fa["x"], fa["qkt"], fa["vt"]
                prw, smw = [], []

                # Head order is parity-major (0,2,4,6,1,3,5,7) so the even
                # heads' row-tile T0 writes only PSUM bank 0 and the odd
                # heads' T8 only bank 1 — row tiles must not touch the same
                # PSUM bank concurrently.
                def sT_block(w):
                    psc = pt.tile([128, H, 128], F32, tag="t")
                    for i in range(H):
                        h = 2 * (i % 4) + i // 4    # slot i holds head h
                        pb = (i // 4) * 64
                        lq = qkt[pb:pb + 64, h // 2, w * W:(w + 1) * W]
                        lk = qkt[pb:pb + 64, 4 + h // 2, w * W:(w + 1) * W]
                        nc.tensor.matmul(psc[:, i, :], lk, lq, start=True,
                                         stop=True, tile_position=(pb, 0))
                    pr = sp4.tile([128, H, 128], BF, tag="pr")
                    nc.scalar.activation(pr[:], psc[:], AF.Exp)
                    prw.append(pr)

                def sums_block(w):
                    # Column-tiled sums: partitions 0-63 get the even heads'
                    # (slots 0-3) sums, 64-127 the odd heads' — matching the
                    # packed attnT layout, so one [128,512] reciprocal (half
                    # the elements) scales the evacuation directly.
                    pr = prw[w]
                    sm = p1.tile([128, QW, 128], F32, tag="m")
                    nc.tensor.matmul(sm[0:64, :, :], on128[:, 0:64],
                                     pr[:, 0:4, :], start=True, stop=True,
                                     tile_position=(0, 0))
                    nc.tensor.matmul(sm[64:128, :, :], on128[:, 0:64],
                                     pr[:, 4:8, :], start=True, stop=True,
                                     tile_position=(0, 64))
                    smw.append(sm)

                def attn_pv(w):
                    # attnV on the UNNORMALIZED exp scores; the softmax
                    # reciprocal is folded into the PSUM evacuation (attn_out)
                    # so the PE never waits on recip.
                    pr, sm = prw[w], smw[w]
                    rcp = sp.tile([128, QW, 128], F32, tag="rcp")
                    nc.vector.reciprocal_approx_fast(out=rcp[:], in_=sm[:])
                    pat = p1.tile([128, D], F32, tag="m")
                    for h in range(H):
                        pb = (h % 2) * 64
                        slot = (h % 2) * 4 + h // 2   # pr slot of head h
                        nc.tensor.matmul(
                            pat[pb:pb + 64, (h // 2) * 128:(h // 2 + 1) * 128],
                            vt[:, w, h * HD:(h + 1) * HD], pr[:, slot, :],
                            start=True, stop=True, tile_position=(0, pb))
                    return rcp, pat

                def attn_out(w, rcp, pat):
                    ats = sp4.tile([128, D], BF, tag="ats")
                    nc.vector.tensor_mul(ats[:], pat[:], rcp[:])
                    pao = p1.tile([128, D], F32, tag="m")
                    for k in range(KC):
                        nc.tensor.matmul(pao[:], ats[:, k * 128:(k + 1) * 128],
                                         wo[:, k, :], start=(k == 0),
                                         stop=(k == KC - 1 and 'ob' not in fl))
                    if 'ob' in fl:
                        nc.tensor.matmul(pao[:], ones1[:, 0:128], obr[:],
                                         start=False, stop=True)
                    return pao

                # Per-window software pipeline: scoresT runs 2 windows ahead
                # of consumption, sums 1 ahead, and quad q-1's FF1 blocks are
                # interleaved as PE filler (cursor-scheduled) so the array
                # never waits on the exp/recip/evacuation/residual chains.
                ffc = [0]

                def ff1_take(n, eng='mix'):
                    if prev is not None and ffc[0] < FC:
                        m1 = min(ffc[0] + n, FC)
                        ff1_blocks(prev, ffc[0], m1, eng)
                        ffc[0] = m1

                sT_block(0); sT_block(1)
                ff1_take(4)            # cover exp(w0)/qkt evacs before sums
                sums_block(0)

                st1 = sp.tile([128, 64], F32, tag="st1")
                y1w = []
                sq = sp.tile([128, D], BF, tag="sq")
                ln1 = sp3.tile([128, QW, D], BF, tag="ln1")
                for w in range(QW):
                    rcp, pat = attn_pv(w)
                    if w + 2 < QW:
                        sT_block(w + 2)
                    if w + 1 < QW:
                        sums_block(w + 1)
                    ff1_take(1)
                    pao = attn_out(w, rcp, pat)
                    y1 = sp4.tile([128, D], F32, tag="y1")
                    nc.vector.scalar_tensor_tensor(
                        y1[:], pao[:], 0.0, x[:, w, :], ALU.add, ALU.add,
                        accum_out=st1[:, w:w + 1])
                    nc.scalar.activation(sq[:], y1[:], AF.Square,
                                         accum_out=st1[:, QW + w:QW + w + 1])
                    y1w.append(y1)
                    ff1_take(2)
                ln_stats(st1)
                gb = (g1b, bb1) if 'gb1' in fl else None
                for w in range(QW):
                    ln_apply(ln1[:, w, :], y1w[w][:], st1, w, gb, 'pool')
                return {"q": q, "ln1": ln1, "xtp_next": fa["xtp_next"]}

            def lnT(state):
                """transpose LN1 output for the FF stage (emitted a full
                stage after LN1-apply so the PE never waits on it)."""
                ln1 = state["ln1"]
                lts = pt.tile([128, KC, QW, 128], BF, tag="t")
                lnt = sp.tile([128, KC, QT], BF, tag="lnt")
                for k in range(KC):
                    for w in range(QW):
                        nc.tensor.transpose(lts[:, k, w, :],
                                            ln1[:, w, k * 128:(k + 1) * 128],
                                            ident[:])
                    nc.vector.tensor_copy(lnt[:, k, :], lts[:, k, :, :])
                state["lnt"] = lnt

            def ff1_blocks(state, m0, m1, eng='mix'):
                """FF1 f-blocks [m0, m1) + relu for quad `state` (uses lnt
                from the lnT stage). `eng` picks the relu engine: the
                mid-window pair goes to Act (its queue is short there, and
                the next window's attnV PSUM allocation WAR-waits on that
                relu via the ring), the end pair to DVE."""
                lnt = state["lnt"]
                if "h1r" not in state:
                    h1r = sp.tile([128, FC, QT], BF, tag="h1r")
                    state["h1r"] = h1r
                h1r = state["h1r"]
                for m in range(m0, m1):
                    ph = p1.tile([128, QT], F32, tag="m")
                    for k in range(KC):
                        nc.tensor.matmul(
                            ph[:], w1t[:, k, m * 128:(m + 1) * 128],
                            lnt[:, k, :], start=(k == 0), stop=(k == KC - 1))
                    use_act = (m % 2 == 1) if eng == 'mix' else (eng == 'act')
                    bm = b1t[:, m:m + 1] if 'b1' in fl else None
                    if use_act:
                        if bm is not None:
                            nc.scalar.activation(h1r[:, m, :], ph[:], AF.Relu,
                                                 bias=bm)
                        else:
                            nc.scalar.activation(h1r[:, m, :], ph[:], AF.Relu)
                    else:
                        if bm is not None:
                            nc.vector.tensor_scalar(h1r[:, m, :], ph[:], bm,
                                                    0.0, ALU.add, ALU.max)
                        else:
                            nc.vector.tensor_scalar(h1r[:, m, :], ph[:], 0.0,
                                                    None, ALU.max)

            def back_b(state, last=False):
                """FF2, LN2, store for quad q. On the final quad the LN2
                applies run on DVE/Act (faster + parallel) instead of Pool:
                there is no PE work left to hide the tail behind."""
                q, ln1, h1r = state["q"], state["ln1"], state["h1r"]
                st2 = sp.tile([128, 64], F32, tag="st2")
                y2w = []
                sq2 = sp.tile([128, D], BF, tag="sq2")
                for w in range(QW):
                    pf = p1.tile([128, D], F32, tag="m")
                    for m in range(FC):
                        nc.tensor.matmul(
                            pf[:], h1r[:, m, w * W:(w + 1) * W],
                            w2t[:, m, :], start=(m == 0),
                            stop=(m == FC - 1 and 'b2' not in fl))
                    if 'b2' in fl:
                        nc.tensor.matmul(pf[:], ones1[:, 0:128], b2r[:],
                                         start=False, stop=True)
                    y2 = sp4.tile([128, D], F32, tag="y2")
                    nc.vector.scalar_tensor_tensor(
                        y2[:], pf[:], 0.0, ln1[:, w, :], ALU.add, ALU.add,
                        accum_out=st2[:, w:w + 1])
                    nc.scalar.activation(sq2[:], y2[:], AF.Square,
                                         accum_out=st2[:, QW + w:QW + w + 1])
                    y2w.append(y2)
                    if last:
                        # per-window stats+apply+store: each window's LN2
                        # chain hides under the next window's FF2 matmuls
                        # instead of piling up after the last one.
                        ln_stats(st2, off=w, width=1)
                        yo = sp.tile([128, D], F32, tag="yo")
                        ln_apply(yo[:], y2[:], st2, w,
                                 (g2b, bb2) if 'gb2' in fl else None,
                                 'act' if w % 2 else 'dve')
                        if w % 2:
                            nc.gpsimd.dma_start(ov[QW * q + w], yo[:])
                        else:
                            nc.sync.dma_start(ov[QW * q + w], yo[:])
                if last:
                    return
                ln_stats(st2)
                gb = (g2b, bb2) if 'gb2' in fl else None
                for w in range(QW):
                    yo = sp.tile([128, D], F32, tag="yo")
                    ln_apply(yo[:], y2w[w][:], st2, w, gb, 'pool')
                    nc.sync.dma_start(ov[QW * q + w], yo[:])

            xq = [load(0)]
            nc.sync.dma_start(on128[:], on128_d[:])
            for k in range(KC):
                nc.sync.dma_start(wqk[:, k, :], wqk_d[:, k * 1024:(k + 1) * 1024])
            nc.sync.dma_start(wv[:], wv_d[:])
            nc.sync.dma_start(wo[:], wo_d[:])
            if n_quads > 1:
                xq.append(load(1))
            nc.sync.dma_start(w1t[:], w1_d[:])
            nc.sync.dma_start(w2t[:], w2_d[:])
            # Pipeline rotation: front_a(q+1) is emitted between back_b(q-1)
            # and lnT(q), so the LN1-apply chain of quad q has back_b + a full
            # QKV stage (~25us of PE work) of cover before the lnT transposes
            # need it.
            xtp_cur = xT(xq[0])
            fa = front_a(0, xq[0], xtp_cur,
                         xq[1] if n_quads > 1 else None)
            pending = None
            for q in range(n_quads):
                nxt = front_b(fa, pending)
                if q + 2 < n_quads:
                    xq.append(load(q + 2))
                if pending is not None:
                    back_b(pending)
                if q + 1 < n_quads:
                    x_next = xq[q + 2] if q + 2 < n_quads else None
                    fa = front_a(q + 1, xq[q + 1], fa["xtp_next"], x_next)
                lnT(nxt)
                pending = nxt
            ff1_blocks(pending, 0, FC)
            back_b(pending, last=True)

    nc.compile()
    return nc


def _pack(wT, kc):
    """[kc*128, N] -> [128, kc*N] with partition p, block k = wT[k*128+p]."""
    n = wT.shape[1]
    return np.ascontiguousarray(
        wT.reshape(kc, 128, n).transpose(1, 0, 2).reshape(128, kc * n))


_CACHE = {}


def _get_nc(n_quads=QUADS, flags=()):
    key = (n_quads, tuple(sorted(flags)))
    if key not in _CACHE:
        _CACHE[key] = _build_nc(n_quads, flags)
    return _CACHE[key]


def _flags(in_proj_b, out_b, ln1_g, ln1_b, b1, b2, ln2_g, ln2_b):
    fl = []
    in_proj_b = np.asarray(in_proj_b)
    if np.any(in_proj_b[:2 * D]):
        fl.append('qkb')
    if np.any(in_proj_b[2 * D:]):
        fl.append('vb')
    if np.any(np.asarray(out_b)):
        fl.append('ob')
    if np.any(np.asarray(b1)):
        fl.append('b1')
    if np.any(np.asarray(b2)):
        fl.append('b2')
    if np.any(np.asarray(ln1_b)) or not np.all(np.asarray(ln1_g) == 1.0):
        fl.append('gb1')
    if np.any(np.asarray(ln2_b)) or not np.all(np.asarray(ln2_g) == 1.0):
        fl.append('gb2')
    return tuple(fl)


def _prep_inputs(src, in_proj_w, in_proj_b, out_w, out_b, ln1_g, ln1_b,
                 w1, b1, w2, b2, ln2_g, ln2_b, n_quads=QUADS, flags=()):
    fl = set(flags)
    src = np.asarray(src, np.float32)
    scale = 1.0 / np.sqrt(HD)

    wqkT = np.asarray(in_proj_w[:2 * D], np.float32).T.copy()   # [512, 1024]
    wqkT[:, :D] *= scale

    common = {
        "wqk": _pack(wqkT.astype(BF16), KC),
        "wv": _pack(np.asarray(in_proj_w[2 * D:], np.float32).T.astype(BF16), KC),
        "wo": _pack(np.asarray(out_w, np.float32).T.astype(BF16), KC),
        "w1t": _pack(np.asarray(w1, np.float32).T.astype(BF16), KC),
        "w2t": _pack(np.asarray(w2, np.float32).T.astype(BF16), FC),
        "ident": np.eye(128, dtype=BF16),
        "ones128": np.ones((128, 128), BF16),
    }
    if 'b1' in fl:
        common["b1t"] = np.ascontiguousarray(
            np.asarray(b1, np.float32).reshape(FC, 128).T)
    if fl & {'qkb', 'vb', 'ob', 'b2'}:
        common["ones1"] = np.ones((1, 512), BF16)
    if 'qkb' in fl:
        bqk = np.asarray(in_proj_b[:2 * D], np.float32).copy()
        bqk[:D] *= scale
        common["qkb"] = bqk.astype(BF16)[None, :]
    if 'vb' in fl:
        common["vbr"] = np.asarray(in_proj_b[2 * D:], np.float32).astype(BF16)[None, :]
    if 'ob' in fl:
        common["obr"] = np.asarray(out_b, np.float32).astype(BF16)[None, :]
    if 'b2' in fl:
        common["b2r"] = np.asarray(b2, np.float32).astype(BF16)[None, :]
    if 'gb1' in fl:
        common["g1b"] = np.ascontiguousarray(np.broadcast_to(
            np.asarray(ln1_g, np.float32), (128, D)))
        common["bb1"] = np.ascontiguousarray(np.broadcast_to(
            np.asarray(ln1_b, np.float32), (128, D)))
    if 'gb2' in fl:
        common["g2b"] = np.ascontiguousarray(np.broadcast_to(
            np.asarray(ln2_g, np.float32), (128, D)))
        common["bb2"] = np.ascontiguousarray(np.broadcast_to(
            np.asarray(ln2_b, np.float32), (128, D)))

    wins = src.reshape(NW_TOT, W, D)
    wpc = n_quads * QW
    in_maps = []
    for c in range(N_CORES):
        m = dict(common)
        m["x"] = np.ascontiguousarray(
            wins[c * wpc:(c + 1) * wpc].reshape(wpc * W, D)).astype(BF16)
        in_maps.append(m)
    return in_maps


def kernel(src, in_proj_w, in_proj_b, out_w, out_b, ln1_g, ln1_b,
           w1, b1, w2, b2, ln2_g, ln2_b):
    flags = _flags(in_proj_b, out_b, ln1_g, ln1_b, b1, b2, ln2_g, ln2_b)
    nc = _get_nc(QUADS, flags)
    in_maps = _prep_inputs(src, in_proj_w, in_proj_b, out_w, out_b, ln1_g,
                           ln1_b, w1, b1, w2, b2, ln2_g, ln2_b, QUADS, flags)
    res = run_bass_kernel_spmd(nc, in_maps, list(range(N_CORES)))
    out = np.concatenate([res.results[c]["out"] for c in range(N_CORES)], axis=0)
    return np.ascontiguousarray(out.reshape(B, S, D)).astype(np.float32)
